# revision 9
# baseline (speedup 1.0000x reference)
"""Trainium2 Bass kernel for cross-attention + entmax15 (sparse attention scores).

Computes, per batch b:
    Q = x_c[b] @ Wq.T + bq ; K = x_n[b] @ Wk.T + bk
    A = Q @ K.T / sqrt(128) ; out[b] = entmax15(A)   (exact 1.5-entmax, row-wise)

Strategy: data-parallel over batch across 8 NeuronCores (B == 8 -> one batch
per core). entmax15 is computed without sorting: the threshold tau* per row
solves f(tau) = sum_i relu(z_i - tau)^2 = 1 (z = A/2), which we solve with a
Gaussian-moment-based analytic initialization + a few safeguarded Newton
iterations (f is convex, decreasing and piecewise quadratic in tau), then
out = relu(z - tau*)^2.
"""

import sys

sys.path.insert(0, "/opt/trn_rl_repo")

import numpy as np

import concourse.bass as bass
import concourse.mybir as mybir
from concourse import bacc
from concourse.bass_utils import run_bass_kernel_spmd
from concourse.masks import make_identity
from concourse.tile import TileContext

B, N, D = 8, 2048, 128
P = 128
NT = N // P  # 16 row-tiles of 128 rows per core
SC = float(1.0 / (2.0 * np.sqrt(np.float64(D))))  # folds /sqrt(D) and /2 into Q
BLK = 512  # columns used for cheap per-row variance / max estimates
L_NEWTON = 3
MARGIN = 0.02
DOWNCLIP = 0.08
GINIT_STEPS = 4
C1 = float(1.0 / np.sqrt(2.0 * np.pi))
# Zelen & Severo (A&S 26.2.16) rational approx of the normal tail:
# Phic(t) ~= phi(t) * (ZB1*k + ZB2*k^2 + ZB3*k^3), k = 1/(1+ZP*t)
ZB1, ZB2, ZB3, ZP = 0.4361836, -0.1201676, 0.9372980, 0.33267

F32 = mybir.dt.float32
F16 = mybir.dt.float16
BF16 = mybir.dt.bfloat16
Alu = mybir.AluOpType
Act = mybir.ActivationFunctionType

_CACHE = {}


def _build_nc() -> bass.Bass:
    nc = bacc.Bacc(None, target_bir_lowering=False)
    xc_d = nc.dram_tensor("x_c", [N, D], F32, kind="ExternalInput")
    xn_d = nc.dram_tensor("x_n", [N, D], F32, kind="ExternalInput")
    wq_d = nc.dram_tensor("Wq", [D, D], F32, kind="ExternalInput")
    bq_d = nc.dram_tensor("bq", [D, 1], F32, kind="ExternalInput")
    wk_d = nc.dram_tensor("Wk", [D, D], F32, kind="ExternalInput")
    bk_d = nc.dram_tensor("bk", [D, 1], F32, kind="ExternalInput")
    out_d = nc.dram_tensor("out", [N, N], F32, kind="ExternalOutput")

    V = nc.vector
    S = nc.scalar
    TE = nc.tensor
    SY = nc.sync

    with TileContext(nc) as tc:
        with (
            tc.tile_pool(name="consts", bufs=1) as consts,
            tc.tile_pool(name="persist", bufs=1) as persist,
            tc.tile_pool(name="stats", bufs=1) as stats,
            tc.tile_pool(name="xin", bufs=4) as xin,
            tc.tile_pool(name="work16", bufs=3) as work16,
            tc.tile_pool(name="junk16", bufs=3) as junk16,
            tc.tile_pool(name="relu32", bufs=3) as relu32,
            tc.tile_pool(name="outp", bufs=3) as outp,
            tc.tile_pool(name="ps", bufs=2, space="PSUM") as ps,
        ):
            ident = consts.tile([P, P], F32, tag="ident")
            make_identity(nc, ident)

            # ---- biases ----
            bq_sb = consts.tile([P, 1], F32, tag="bq")
            bk_sb = consts.tile([P, 1], F32, tag="bk")
            SY.dma_start(out=bq_sb[:, :], in_=bq_d[:, :])
            SY.dma_start(out=bk_sb[:, :], in_=bk_d[:, :])
            bqs = consts.tile([P, 1], F32, tag="bqs")
            V.tensor_scalar(bqs[:, :], bq_sb[:, :], SC, None, Alu.mult)

            # ---- weights + their transposes (lhsT = W^T) ----
            wq_sb = consts.tile([P, P], F32, tag="wq")
            wk_sb = consts.tile([P, P], F32, tag="wk")
            SY.dma_start(out=wq_sb[:, :], in_=wq_d[:, :])
            SY.dma_start(out=wk_sb[:, :], in_=wk_d[:, :])
            wqT = consts.tile([P, P], F32, tag="wqT")
            wkT = consts.tile([P, P], F32, tag="wkT")
            wt_ps = ps.tile([P, 2, P], F32, tag="ps")
            TE.transpose(wt_ps[:, 0, :], wq_sb[:, :], ident[:, :])
            TE.transpose(wt_ps[:, 1, :], wk_sb[:, :], ident[:, :])
            V.tensor_copy(wqT[:, :], wt_ps[:, 0, :])
            V.tensor_copy(wkT[:, :], wt_ps[:, 1, :])

            # ---- load x_c, x_n and transpose to [e, n] layout ----
            xcT = persist.tile([P, NT, P], F32, tag="xcT")  # x_c^T  [e, n]
            xnT = persist.tile([P, NT, P], F32, tag="xnT")  # x_n^T  [e, n]
            for src_d, dstT in ((xc_d, xcT), (xn_d, xnT)):
                x_ps = ps.tile([P, NT, P], F32, tag="ps")
                for j in range(NT):
                    xt = xin.tile([P, P], F32, tag="xin")
                    SY.dma_start(out=xt[:, :], in_=src_d[j * P : (j + 1) * P, :])
                    TE.transpose(x_ps[:, j, :], xt[:, :], ident[:, :])
                S.activation(dstT[:, :, :], x_ps[:, :, :], Act.Copy)

            # ---- projections: Q'^T = (Wq x_c^T + bq) * SC, K'^T = Wk x_n^T + bk
            # bf16 outputs feed the big QK^T matmul.
            QTb = persist.tile([P, N], BF16, tag="QTb")
            KTb = persist.tile([P, N], BF16, tag="KTb")
            for (wT, xT, dst, bias_ap, scale) in (
                (wqT, xcT, QTb, bqs, SC),
                (wkT, xnT, KTb, bk_sb, 1.0),
            ):
                pr_ps = ps.tile([P, N], F32, tag="ps")
                for mb in range(4):
                    TE.matmul(
                        pr_ps[:, mb * 512 : (mb + 1) * 512],
                        lhsT=wT[:, :],
                        rhs=xT[:, 4 * mb : 4 * mb + 4, :],
                        start=True,
                        stop=True,
                    )
                S.activation(
                    dst[:, :], pr_ps[:, :], Act.Identity, bias=bias_ap[:, :], scale=scale
                )

            # ---- per-row stat tiles [P, NT] fp32 ----
            def st(tag):
                return stats.tile([P, NT], F32, tag=tag, name=tag)

            mu_a, s2_a, msub = st("mu"), st("s2"), st("msub")
            var, sig, t_ = st("var"), st("sig"), st("t")
            tsq, e_, r_, phic = st("tsq"), st("e"), st("r"), st("phic")
            w_, f_f, tp1, tp2 = st("w"), st("ff"), st("tp1"), st("tp2")
            ns2, rden = st("ns2"), st("rden")
            tau, lo_, hi_ = st("tau"), st("lo"), st("hi")
            ab, g_a, f_a, ntau = st("ab"), st("g"), st("f"), st("ntau")
            mid, tna, tnb, dd = st("mid"), st("tna"), st("tnb"), st("dd")

            # ---- first z pass: z = Q'^T.T @ K'^T  (tile j = rows 128j..128j+127)
            # copy to fp16 (for cheap Newton passes) + row stats
            z16 = persist.tile([P, NT, N], F16, tag="z16")
            for j in range(NT):
                z_ps = ps.tile([P, N], F32, tag="ps")
                for mb in range(4):
                    TE.matmul(
                        z_ps[:, mb * 512 : (mb + 1) * 512],
                        lhsT=QTb[:, j * P : (j + 1) * P],
                        rhs=KTb[:, mb * 512 : (mb + 1) * 512],
                        start=True,
                        stop=True,
                    )
                if j % 3 == 2:
                    # DVE copy+cast with mean accumulate (op1 = reduce op)
                    V.tensor_scalar(
                        z16[:, j, :], z_ps[:, :], 0.0, None, Alu.add, Alu.add,
                        accum_out=mu_a[:, j : j + 1],
                    )
                else:
                    S.activation(
                        z16[:, j, :], z_ps[:, :], Act.Identity,
                        accum_out=mu_a[:, j : j + 1],
                    )
                # block stats on the fp16 copy
                V.tensor_reduce(
                    msub[:, j : j + 1], z16[:, j, 0:BLK], mybir.AxisListType.X, Alu.max
                )
                jk = junk16.tile([P, BLK], F16, tag="jkb")
                S.activation(
                    jk[:, :], z16[:, j, 0:BLK], Act.Square,
                    accum_out=s2_a[:, j : j + 1],
                )

            # ---- Gaussian-moment analytic init for tau ----
            # mu = full-row mean, var = block second moment - mu^2
            V.tensor_scalar(mu_a[:, :], mu_a[:, :], 1.0 / N, None, Alu.mult)
            V.tensor_scalar(s2_a[:, :], s2_a[:, :], 1.0 / BLK, None, Alu.mult)
            V.tensor_tensor(tp1[:, :], mu_a[:, :], mu_a[:, :], Alu.mult)
            V.tensor_tensor(var[:, :], s2_a[:, :], tp1[:, :], Alu.subtract)
            V.tensor_scalar(var[:, :], var[:, :], 1e-12, None, Alu.max)
            S.activation(sig[:, :], var[:, :], Act.Sqrt)
            # ns2 = N * var * C1 (C1 folded so phi == exp(-t^2/2) below)
            V.tensor_scalar(ns2[:, :], var[:, :], float(N) * C1, None, Alu.mult)
            # Solve N*sig^2*F(t) = 1,  F(t) = (1+t^2)*Phic(t) - t*phi(t), by
            # Newton in t, with Phic via the Zelen-Severo rational approx.
            V.memset(t_[:, :], 2.0)
            for _ in range(GINIT_STEPS):
                V.tensor_tensor(tsq[:, :], t_[:, :], t_[:, :], Alu.mult)
                S.activation(e_[:, :], tsq[:, :], Act.Exp, scale=-0.5)  # phi/C1
                V.tensor_scalar(tp1[:, :], t_[:, :], ZP, 1.0, Alu.mult, Alu.add)
                V.reciprocal(r_[:, :], tp1[:, :])  # k = 1/(1+ZP*t)
                V.tensor_scalar(phic[:, :], r_[:, :], ZB3, ZB2, Alu.mult, Alu.add)
                V.tensor_tensor(phic[:, :], phic[:, :], r_[:, :], Alu.mult)
                V.tensor_scalar(phic[:, :], phic[:, :], ZB1, None, Alu.add)
                V.tensor_tensor(phic[:, :], phic[:, :], r_[:, :], Alu.mult)  # Rk
                V.tensor_scalar(w_[:, :], tsq[:, :], 1.0, None, Alu.add)  # 1+t^2
                V.tensor_tensor(f_f[:, :], w_[:, :], phic[:, :], Alu.mult)
                V.tensor_tensor(f_f[:, :], f_f[:, :], t_[:, :], Alu.subtract)
                V.tensor_tensor(f_f[:, :], f_f[:, :], e_[:, :], Alu.mult)  # F/C1
                V.tensor_tensor(tp2[:, :], t_[:, :], phic[:, :], Alu.mult)
                V.tensor_scalar(tp2[:, :], tp2[:, :], -1.0, None, Alu.add)
                V.tensor_tensor(tp2[:, :], tp2[:, :], e_[:, :], Alu.mult)  # dF/(2 C1)
                # num = ns2*F - 1 ; den = ns2*(dF/2) ; t -= num/(2*den)
                V.tensor_tensor(tp1[:, :], f_f[:, :], ns2[:, :], Alu.mult)
                V.tensor_scalar(tp1[:, :], tp1[:, :], -1.0, None, Alu.add)
                V.tensor_tensor(tp2[:, :], tp2[:, :], ns2[:, :], Alu.mult)
                V.reciprocal(rden[:, :], tp2[:, :])
                V.tensor_tensor(tp1[:, :], tp1[:, :], rden[:, :], Alu.mult)
                V.scalar_tensor_tensor(
                    t_[:, :], tp1[:, :], -0.5, t_[:, :], Alu.mult, Alu.add
                )
                V.tensor_scalar(t_[:, :], t_[:, :], 0.0, 6.0, Alu.max, Alu.min)
            # tau0 = max(mu + sig*t - MARGIN, msub - 1)
            V.tensor_tensor(tp1[:, :], sig[:, :], t_[:, :], Alu.mult)
            V.tensor_tensor(tau[:, :], mu_a[:, :], tp1[:, :], Alu.add)
            V.tensor_scalar(tau[:, :], tau[:, :], -MARGIN, None, Alu.add)
            V.tensor_scalar(lo_[:, :], msub[:, :], -1.0, None, Alu.add)
            V.tensor_scalar(hi_[:, :], msub[:, :], 1.0, None, Alu.add)
            V.tensor_tensor(tau[:, :], tau[:, :], lo_[:, :], Alu.max)

            # ---- safeguarded Newton iterations on fp16 z ----
            for it in range(L_NEWTON):
                # ACT: T = relu(z - tau) with accum -> g ; DVE: T*T accum -> f
                V.tensor_scalar(ntau[:, :], tau[:, :], -1.0, None, Alu.mult)
                for j in range(NT):
                    t16 = work16.tile([P, N], F16, tag="T16")
                    S.activation(
                        t16[:, :], z16[:, j, :], Act.Relu,
                        bias=ntau[:, j : j + 1],
                        accum_out=g_a[:, j : j + 1],
                    )
                    jq = junk16.tile([P, N], F16, tag="jq")
                    if j % 8 == 7:
                        S.activation(
                            jq[:, :], t16[:, :], Act.Square,
                            accum_out=f_a[:, j : j + 1],
                        )
                    else:
                        V.scalar_tensor_tensor(
                            jq[:, :], t16[:, :], 0.0, t16[:, :], Alu.add, Alu.mult,
                            accum_out=f_a[:, j : j + 1],
                        )
                # batched [P, NT] update
                V.tensor_scalar(ab[:, :], f_a[:, :], 1.0, None, Alu.is_ge)
                # LO = max(LO, tau - 4*(1-ab)) ; HI = min(HI, tau + 4*ab)
                V.tensor_scalar(tp1[:, :], ab[:, :], -1.0, None, Alu.add)  # ab-1
                V.scalar_tensor_tensor(
                    tp1[:, :], tp1[:, :], 4.0, tau[:, :], Alu.mult, Alu.add
                )  # tau + 4*(ab-1)
                V.tensor_tensor(lo_[:, :], lo_[:, :], tp1[:, :], Alu.max)
                V.scalar_tensor_tensor(
                    tp2[:, :], ab[:, :], 4.0, tau[:, :], Alu.mult, Alu.add
                )  # tau + 4*ab
                V.tensor_tensor(hi_[:, :], hi_[:, :], tp2[:, :], Alu.min)
                # tn = tau + max(0.5*(f-1)/g, -DOWNCLIP)
                V.tensor_scalar(tp1[:, :], g_a[:, :], 1e-20, None, Alu.max)
                V.reciprocal(rden[:, :], tp1[:, :])
                V.tensor_scalar(tp2[:, :], f_a[:, :], -1.0, None, Alu.add)
                V.tensor_tensor(tp2[:, :], tp2[:, :], rden[:, :], Alu.mult)
                V.tensor_scalar(
                    tp2[:, :], tp2[:, :], 0.5, -DOWNCLIP, Alu.mult, Alu.max
                )
                V.tensor_tensor(tp2[:, :], tau[:, :], tp2[:, :], Alu.add)
                # mid = 0.5*(LO+HI)
                V.tensor_tensor(mid[:, :], lo_[:, :], hi_[:, :], Alu.add)
                V.tensor_scalar(mid[:, :], mid[:, :], 0.5, None, Alu.mult)
                # tau = ab ? min(tn, HI) : max(tn, mid)
                V.tensor_tensor(tna[:, :], tp2[:, :], hi_[:, :], Alu.min)
                V.tensor_tensor(tnb[:, :], tp2[:, :], mid[:, :], Alu.max)
                V.tensor_tensor(dd[:, :], tna[:, :], tnb[:, :], Alu.subtract)
                V.tensor_tensor(dd[:, :], dd[:, :], ab[:, :], Alu.mult)
                V.tensor_tensor(tau[:, :], tnb[:, :], dd[:, :], Alu.add)

            # ---- output pass: recompute z (same bf16 matmul), out = relu(z-tau)^2
            V.tensor_scalar(ntau[:, :], tau[:, :], -1.0, None, Alu.mult)
            for j in range(NT):
                z_ps = ps.tile([P, N], F32, tag="ps")
                for mb in range(4):
                    TE.matmul(
                        z_ps[:, mb * 512 : (mb + 1) * 512],
                        lhsT=QTb[:, j * P : (j + 1) * P],
                        rhs=KTb[:, mb * 512 : (mb + 1) * 512],
                        start=True,
                        stop=True,
                    )
                tr = relu32.tile([P, N], F32, tag="tr")
                S.activation(
                    tr[:, :], z_ps[:, :], Act.Relu, bias=ntau[:, j : j + 1]
                )
                ou = outp.tile([P, N], F32, tag="ou")
                V.tensor_tensor(ou[:, :], tr[:, :], tr[:, :], Alu.mult)
                SY.dma_start(out=out_d[j * P : (j + 1) * P, :], in_=ou[:, :])

    nc.compile()
    return nc


def _get_nc() -> bass.Bass:
    if "nc" not in _CACHE:
        _CACHE["nc"] = _build_nc()
    return _CACHE["nc"]


def _run(in_maps, trace=False, **kw):
    nc = _get_nc()
    return run_bass_kernel_spmd(
        nc, in_maps, core_ids=list(range(B)), trace=trace, **kw
    )


def _make_in_maps(x_c, x_n, Wq, bq, Wk, bk):
    x_c = np.ascontiguousarray(np.asarray(x_c, dtype=np.float32))
    x_n = np.ascontiguousarray(np.asarray(x_n, dtype=np.float32))
    Wq = np.ascontiguousarray(np.asarray(Wq, dtype=np.float32))
    Wk = np.ascontiguousarray(np.asarray(Wk, dtype=np.float32))
    bq = np.ascontiguousarray(np.asarray(bq, dtype=np.float32).reshape(D, 1))
    bk = np.ascontiguousarray(np.asarray(bk, dtype=np.float32).reshape(D, 1))
    return [
        {
            "x_c": x_c[i],
            "x_n": x_n[i],
            "Wq": Wq,
            "bq": bq,
            "Wk": Wk,
            "bk": bk,
        }
        for i in range(B)
    ]


def kernel(x_c, x_n, Wq, bq, Wk, bk):
    res = _run(_make_in_maps(x_c, x_n, Wq, bq, Wk, bk))
    out = np.stack([res.results[i]["out"] for i in range(B)], axis=0)
    return out.astype(np.float32)


if __name__ == "__main__":
    rng = np.random.default_rng(0)
    inputs = {
        "x_c": rng.standard_normal((B, N, D)).astype(np.float32),
        "x_n": rng.standard_normal((B, N, D)).astype(np.float32),
        "Wq": rng.uniform(-SC, SC, (D, D)).astype(np.float32),
        "bq": rng.uniform(-SC, SC, (D,)).astype(np.float32),
        "Wk": rng.uniform(-SC, SC, (D, D)).astype(np.float32),
        "bk": rng.uniform(-SC, SC, (D,)).astype(np.float32),
    }
    out = kernel(**inputs)
    print("out", out.shape, out.dtype, float(out.max()))


# revision 11
# speedup vs baseline: 1.1885x; 1.1885x over previous
"""Trainium2 Bass kernel for cross-attention + entmax15 (sparse attention scores).

Computes, per batch b:
    Q = x_c[b] @ Wq.T + bq ; K = x_n[b] @ Wk.T + bk
    A = Q @ K.T / sqrt(128) ; out[b] = entmax15(A)   (exact 1.5-entmax, row-wise)

Strategy: data-parallel over batch across 8 NeuronCores (B == 8 -> one batch
per core). entmax15 is computed without sorting: the threshold tau* per row
solves f(tau) = sum_i relu(z_i - tau)^2 = 1 (z = A/2).  f is convex,
decreasing, piecewise quadratic in tau.  We initialize tau analytically from
per-row Gaussian moment estimates, refine with three f-only evaluations
(bracketed secant steps), then solve the local quadratic through the last
three (tau, f) points (Muller step — exact on a fixed support piece).
Finally out = relu(z - tau)^2.
"""

import sys

sys.path.insert(0, "/opt/trn_rl_repo")

import numpy as np

import concourse.bass as bass
import concourse.mybir as mybir
from concourse import bacc
from concourse.bass_utils import run_bass_kernel_spmd
from concourse.masks import make_identity
from concourse.tile import TileContext

B, N, D = 8, 2048, 128
P = 128
NT = N // P  # 16 row-tiles of 128 rows per core
SC = float(1.0 / (2.0 * np.sqrt(np.float64(D))))  # folds /sqrt(D) and /2 into Q
BLK = 512  # columns used for cheap per-row variance / max estimates
MARGIN = 0.02
DOWNCLIP = 0.08
GINIT_STEPS = 4
C1 = float(1.0 / np.sqrt(2.0 * np.pi))
# Zelen & Severo (A&S 26.2.16) rational approx of the normal tail:
# Phic(t) ~= phi(t) * (ZB1*k + ZB2*k^2 + ZB3*k^3), k = 1/(1+ZP*t)
ZB1, ZB2, ZB3, ZP = 0.4361836, -0.1201676, 0.9372980, 0.33267

F32 = mybir.dt.float32
F16 = mybir.dt.float16
BF16 = mybir.dt.bfloat16
Alu = mybir.AluOpType
Act = mybir.ActivationFunctionType

_CACHE = {}


def _build_nc() -> bass.Bass:
    nc = bacc.Bacc(None, target_bir_lowering=False)
    xc_d = nc.dram_tensor("x_c", [N, D], F32, kind="ExternalInput")
    xn_d = nc.dram_tensor("x_n", [N, D], F32, kind="ExternalInput")
    wq_d = nc.dram_tensor("Wq", [D, D], F32, kind="ExternalInput")
    bq_d = nc.dram_tensor("bq", [D, 1], F32, kind="ExternalInput")
    wk_d = nc.dram_tensor("Wk", [D, D], F32, kind="ExternalInput")
    bk_d = nc.dram_tensor("bk", [D, 1], F32, kind="ExternalInput")
    out_d = nc.dram_tensor("out", [N, N], F32, kind="ExternalOutput")

    V = nc.vector
    S = nc.scalar
    TE = nc.tensor
    SY = nc.sync

    with TileContext(nc) as tc:
        with (
            tc.tile_pool(name="consts", bufs=1) as consts,
            tc.tile_pool(name="persist", bufs=1) as persist,
            tc.tile_pool(name="stats", bufs=1) as stats,
            tc.tile_pool(name="work16", bufs=3) as work16,
            tc.tile_pool(name="junk16", bufs=3) as junk16,
            tc.tile_pool(name="outp", bufs=3) as outp,
            tc.tile_pool(name="ps", bufs=2, space="PSUM") as ps,
        ):
            ident = consts.tile([P, P], F32, tag="ident")
            make_identity(nc, ident)

            # ---- biases ----
            bq_sb = consts.tile([P, 1], F32, tag="bq")
            bk_sb = consts.tile([P, 1], F32, tag="bk")
            SY.dma_start(out=bq_sb[:, :], in_=bq_d[:, :])
            SY.dma_start(out=bk_sb[:, :], in_=bk_d[:, :])
            bqs = consts.tile([P, 1], F32, tag="bqs")
            V.tensor_scalar(bqs[:, :], bq_sb[:, :], SC, None, Alu.mult)

            # ---- weights + their transposes (lhsT = W^T) ----
            wq_sb = consts.tile([P, P], F32, tag="wq")
            wk_sb = consts.tile([P, P], F32, tag="wk")
            SY.dma_start(out=wq_sb[:, :], in_=wq_d[:, :])
            SY.dma_start(out=wk_sb[:, :], in_=wk_d[:, :])
            wqT = consts.tile([P, P], F32, tag="wqT")
            wkT = consts.tile([P, P], F32, tag="wkT")
            wt_ps = ps.tile([P, 2, P], F32, tag="ps")
            TE.transpose(wt_ps[:, 0, :], wq_sb[:, :], ident[:, :])
            TE.transpose(wt_ps[:, 1, :], wk_sb[:, :], ident[:, :])
            V.tensor_copy(wqT[:, :], wt_ps[:, 0, :])
            V.tensor_copy(wkT[:, :], wt_ps[:, 1, :])

            # ---- load x_c, x_n (4 chunked DMAs each) and transpose to [e, n]
            xc_sb = persist.tile([P, NT, P], F32, tag="xc_sb")
            xn_sb = persist.tile([P, NT, P], F32, tag="xn_sb")
            xcT = persist.tile([P, NT, P], F32, tag="xcT")  # x_c^T  [e, n]
            xnT = persist.tile([P, NT, P], F32, tag="xnT")  # x_n^T  [e, n]
            for src_d, stage, dstT in ((xc_d, xc_sb, xcT), (xn_d, xn_sb, xnT)):
                src_r = src_d.rearrange("(t p) e -> p t e", p=P)
                for c in range(4):
                    SY.dma_start(
                        out=stage[:, 4 * c : 4 * c + 4, :],
                        in_=src_r[:, 4 * c : 4 * c + 4, :],
                    )
                x_ps = ps.tile([P, NT, P], F32, tag="ps")
                for j in range(NT):
                    TE.transpose(x_ps[:, j, :], stage[:, j, :], ident[:, :])
                S.activation(dstT[:, :, :], x_ps[:, :, :], Act.Copy)

            # ---- projections: Q'^T = (Wq x_c^T + bq) * SC, K'^T = Wk x_n^T + bk
            QTb = persist.tile([P, N], BF16, tag="QTb")
            KTb = persist.tile([P, N], BF16, tag="KTb")
            for (wT, xT, dst, bias_ap, scale) in (
                (wqT, xcT, QTb, bqs, SC),
                (wkT, xnT, KTb, bk_sb, 1.0),
            ):
                pr_ps = ps.tile([P, N], F32, tag="ps")
                for mb in range(4):
                    TE.matmul(
                        pr_ps[:, mb * 512 : (mb + 1) * 512],
                        lhsT=wT[:, :],
                        rhs=xT[:, 4 * mb : 4 * mb + 4, :],
                        start=True,
                        stop=True,
                    )
                S.activation(
                    dst[:, :], pr_ps[:, :], Act.Identity, bias=bias_ap[:, :], scale=scale
                )

            # ---- per-row stat tiles [P, NT] fp32 ----
            def st(tag):
                return stats.tile([P, NT], F32, tag=tag, name=tag)

            mu_a, s2_a, msub = st("mu"), st("s2"), st("msub")
            var, sig, t_ = st("var"), st("sig"), st("t")
            tsq, e_, r_, phic = st("tsq"), st("e"), st("r"), st("phic")
            w_, f_f, tp1, tp2 = st("w"), st("ff"), st("tp1"), st("tp2")
            ns2, rden, gmod = st("ns2"), st("rden"), st("gmod")
            lo_, hi_ = st("lo"), st("hi")
            ab = st("ab")
            tau1, tau2, tau3, tau4 = st("tau1"), st("tau2"), st("tau3"), st("tau4")
            f1, f2, f3 = st("f1"), st("f2"), st("f3")
            d12, d23, aq, bq_c, cq = st("d12"), st("d23"), st("aq"), st("bqc"), st("cq")
            mid, sq_ = st("mid"), st("sq")

            # ---- z pass: z = Q'^T.T @ K'^T ; keep fp16 copy + row stats ----
            z16 = persist.tile([P, NT, N], F16, tag="z16")
            for j in range(NT):
                z_ps = ps.tile([P, N], F32, tag="ps")
                for mb in range(4):
                    TE.matmul(
                        z_ps[:, mb * 512 : (mb + 1) * 512],
                        lhsT=QTb[:, j * P : (j + 1) * P],
                        rhs=KTb[:, mb * 512 : (mb + 1) * 512],
                        start=True,
                        stop=True,
                    )
                if j % 2 == 1:
                    # DVE copy+cast with mean accumulate (op1 = reduce op)
                    V.tensor_scalar(
                        z16[:, j, :], z_ps[:, :], 0.0, None, Alu.add, Alu.add,
                        accum_out=mu_a[:, j : j + 1],
                    )
                else:
                    S.activation(
                        z16[:, j, :], z_ps[:, :], Act.Identity,
                        accum_out=mu_a[:, j : j + 1],
                    )
                # block stats on the fp16 copy
                V.tensor_reduce(
                    msub[:, j : j + 1], z16[:, j, 0:BLK], mybir.AxisListType.X, Alu.max
                )
                jk = junk16.tile([P, BLK], F16, tag="jkb")
                S.activation(
                    jk[:, :], z16[:, j, 0:BLK], Act.Square,
                    accum_out=s2_a[:, j : j + 1],
                )

            # ---- Gaussian-moment analytic init for tau ----
            V.tensor_scalar(mu_a[:, :], mu_a[:, :], 1.0 / N, None, Alu.mult)
            V.tensor_scalar(s2_a[:, :], s2_a[:, :], 1.0 / BLK, None, Alu.mult)
            V.tensor_tensor(tp1[:, :], mu_a[:, :], mu_a[:, :], Alu.mult)
            V.tensor_tensor(var[:, :], s2_a[:, :], tp1[:, :], Alu.subtract)
            V.tensor_scalar(var[:, :], var[:, :], 1e-12, None, Alu.max)
            S.activation(sig[:, :], var[:, :], Act.Sqrt)
            # ns2 = N * var * C1 (C1 folded so phi == exp(-t^2/2) below)
            V.tensor_scalar(ns2[:, :], var[:, :], float(N) * C1, None, Alu.mult)
            # Solve N*sig^2*F(t) = 1,  F(t) = (1+t^2)*Phic(t) - t*phi(t), by
            # Newton in t, with Phic via the Zelen-Severo rational approx.
            V.memset(t_[:, :], 2.0)
            for gi in range(GINIT_STEPS + 1):
                V.tensor_tensor(tsq[:, :], t_[:, :], t_[:, :], Alu.mult)
                S.activation(e_[:, :], tsq[:, :], Act.Exp, scale=-0.5)  # phi/C1
                V.tensor_scalar(tp1[:, :], t_[:, :], ZP, 1.0, Alu.mult, Alu.add)
                V.reciprocal(r_[:, :], tp1[:, :])  # k = 1/(1+ZP*t)
                V.tensor_scalar(phic[:, :], r_[:, :], ZB3, ZB2, Alu.mult, Alu.add)
                V.tensor_tensor(phic[:, :], phic[:, :], r_[:, :], Alu.mult)
                V.tensor_scalar(phic[:, :], phic[:, :], ZB1, None, Alu.add)
                V.tensor_tensor(phic[:, :], phic[:, :], r_[:, :], Alu.mult)  # Rk
                # dF/(2 C1) = (t*Rk - 1) * e
                V.tensor_tensor(tp2[:, :], t_[:, :], phic[:, :], Alu.mult)
                V.tensor_scalar(tp2[:, :], tp2[:, :], -1.0, None, Alu.add)
                V.tensor_tensor(tp2[:, :], tp2[:, :], e_[:, :], Alu.mult)
                if gi == GINIT_STEPS:
                    # analytic slope model: g(tau) = -N*sig*C1*(t*Rk-1)*e
                    V.tensor_tensor(gmod[:, :], tp2[:, :], sig[:, :], Alu.mult)
                    V.tensor_scalar(
                        gmod[:, :], gmod[:, :], -float(N) * C1, None, Alu.mult
                    )
                    V.tensor_scalar(gmod[:, :], gmod[:, :], 1e-6, None, Alu.max)
                    break
                V.tensor_scalar(w_[:, :], tsq[:, :], 1.0, None, Alu.add)  # 1+t^2
                V.tensor_tensor(f_f[:, :], w_[:, :], phic[:, :], Alu.mult)
                V.tensor_tensor(f_f[:, :], f_f[:, :], t_[:, :], Alu.subtract)
                V.tensor_tensor(f_f[:, :], f_f[:, :], e_[:, :], Alu.mult)  # F/C1
                # num = ns2*F - 1 ; den = ns2*(dF/2) ; t -= num/(2*den)
                V.tensor_tensor(tp1[:, :], f_f[:, :], ns2[:, :], Alu.mult)
                V.tensor_scalar(tp1[:, :], tp1[:, :], -1.0, None, Alu.add)
                V.tensor_tensor(tp2[:, :], tp2[:, :], ns2[:, :], Alu.mult)
                V.reciprocal(rden[:, :], tp2[:, :])
                V.tensor_tensor(tp1[:, :], tp1[:, :], rden[:, :], Alu.mult)
                V.scalar_tensor_tensor(
                    t_[:, :], tp1[:, :], -0.5, t_[:, :], Alu.mult, Alu.add
                )
                V.tensor_scalar(t_[:, :], t_[:, :], 0.0, 6.0, Alu.max, Alu.min)
            # tau1 = max(mu + sig*t - MARGIN, msub - 1)
            V.tensor_tensor(tp1[:, :], sig[:, :], t_[:, :], Alu.mult)
            V.tensor_tensor(tau1[:, :], mu_a[:, :], tp1[:, :], Alu.add)
            V.tensor_scalar(tau1[:, :], tau1[:, :], -MARGIN, None, Alu.add)
            V.tensor_scalar(lo_[:, :], msub[:, :], -1.0, None, Alu.add)
            V.tensor_scalar(hi_[:, :], msub[:, :], 1.0, None, Alu.add)
            V.tensor_tensor(tau1[:, :], tau1[:, :], lo_[:, :], Alu.max)

            # ---- f-only evaluation: f(tau) = sum relu(z16 - tau)^2 ----
            def eval_f(tau_ap, f_ap):
                for j in range(NT):
                    t16 = work16.tile([P, N], F16, tag="T16", name="t16")
                    V.tensor_scalar(
                        t16[:, :], z16[:, j, :], tau_ap[:, j : j + 1], 0.0,
                        Alu.subtract, Alu.max,
                    )
                    jq = junk16.tile([P, N], F16, tag="jq", name="jq")
                    if j % 3 == 2:
                        V.scalar_tensor_tensor(
                            jq[:, :], t16[:, :], 0.0, t16[:, :], Alu.add, Alu.mult,
                            accum_out=f_ap[:, j : j + 1],
                        )
                    else:
                        S.activation(
                            jq[:, :], t16[:, :], Act.Square,
                            accum_out=f_ap[:, j : j + 1],
                        )

            def update_brackets(tau_ap, f_ap):
                # ab = f >= 1 ; LO = max(LO, tau - 4*(1-ab)) ; HI = min(HI, tau + 4*ab)
                V.tensor_scalar(ab[:, :], f_ap[:, :], 1.0, None, Alu.is_ge)
                V.tensor_scalar(tp1[:, :], ab[:, :], -1.0, None, Alu.add)
                V.scalar_tensor_tensor(
                    tp1[:, :], tp1[:, :], 4.0, tau_ap[:, :], Alu.mult, Alu.add
                )
                V.tensor_tensor(lo_[:, :], lo_[:, :], tp1[:, :], Alu.max)
                V.scalar_tensor_tensor(
                    tp2[:, :], ab[:, :], 4.0, tau_ap[:, :], Alu.mult, Alu.add
                )
                V.tensor_tensor(hi_[:, :], hi_[:, :], tp2[:, :], Alu.min)

            def guarded_step(tau_ap, f_ap, slope_ap, out_ap):
                # tn = tau + max((f-1)/slope, -DOWNCLIP)
                # out = ab ? min(tn, HI) : max(tn, mid)
                V.reciprocal(rden[:, :], slope_ap[:, :])
                V.tensor_scalar(tp2[:, :], f_ap[:, :], -1.0, None, Alu.add)
                V.tensor_tensor(tp2[:, :], tp2[:, :], rden[:, :], Alu.mult)
                V.tensor_scalar(tp2[:, :], tp2[:, :], -DOWNCLIP, None, Alu.max)
                V.tensor_tensor(tp2[:, :], tau_ap[:, :], tp2[:, :], Alu.add)
                V.tensor_tensor(mid[:, :], lo_[:, :], hi_[:, :], Alu.add)
                V.tensor_scalar(mid[:, :], mid[:, :], 0.5, None, Alu.mult)
                V.tensor_tensor(tp1[:, :], tp2[:, :], hi_[:, :], Alu.min)  # above
                V.tensor_tensor(tp2[:, :], tp2[:, :], mid[:, :], Alu.max)  # below
                V.tensor_tensor(tp1[:, :], tp1[:, :], tp2[:, :], Alu.subtract)
                V.tensor_tensor(tp1[:, :], tp1[:, :], ab[:, :], Alu.mult)
                V.tensor_tensor(out_ap[:, :], tp2[:, :], tp1[:, :], Alu.add)

            def force_distinct(tau_ap, other_ap, off=-1e-4):
                # if |tau - other| < 1e-5: tau = other + off
                V.tensor_tensor(tp1[:, :], tau_ap[:, :], other_ap[:, :], Alu.subtract)
                S.activation(tp1[:, :], tp1[:, :], Act.Abs)
                V.tensor_scalar(tp1[:, :], tp1[:, :], 1e-5, None, Alu.is_lt)
                # tau = tau*(1-m) + (other+off)*m  ==  tau + m*(other+off-tau)
                V.tensor_scalar(tp2[:, :], other_ap[:, :], off, None, Alu.add)
                V.tensor_tensor(tp2[:, :], tp2[:, :], tau_ap[:, :], Alu.subtract)
                V.tensor_tensor(tp2[:, :], tp2[:, :], tp1[:, :], Alu.mult)
                V.tensor_tensor(tau_ap[:, :], tau_ap[:, :], tp2[:, :], Alu.add)

            # eval 1: Newton step with analytic slope 2*g_model
            eval_f(tau1, f1)
            update_brackets(tau1, f1)
            V.tensor_scalar(tp1[:, :], gmod[:, :], 2.0, None, Alu.mult)
            V.tensor_copy(sq_[:, :], tp1[:, :])  # slope buffer
            guarded_step(tau1, f1, sq_, tau2)
            force_distinct(tau2, tau1)

            # eval 2: secant slope from (tau1,f1)
            eval_f(tau2, f2)
            update_brackets(tau2, f2)
            # d12 = (f1-f2)/(tau1-tau2)  (negative); slope = max(-d12, eps->gmod)
            V.tensor_tensor(tp1[:, :], tau1[:, :], tau2[:, :], Alu.subtract)
            V.reciprocal(rden[:, :], tp1[:, :])
            V.tensor_tensor(d12[:, :], f1[:, :], f2[:, :], Alu.subtract)
            V.tensor_tensor(d12[:, :], d12[:, :], rden[:, :], Alu.mult)
            V.tensor_scalar(sq_[:, :], d12[:, :], -1.0, None, Alu.mult)
            V.tensor_scalar(sq_[:, :], sq_[:, :], 1e-6, None, Alu.max)
            guarded_step(tau2, f2, sq_, tau3)
            force_distinct(tau3, tau2, off=-1.3e-4)
            force_distinct(tau3, tau1, off=-2.1e-4)

            # eval 3 + Muller quadratic solve through the three points
            eval_f(tau3, f3)
            update_brackets(tau3, f3)
            V.tensor_tensor(tp1[:, :], tau2[:, :], tau3[:, :], Alu.subtract)
            V.reciprocal(rden[:, :], tp1[:, :])
            V.tensor_tensor(d23[:, :], f2[:, :], f3[:, :], Alu.subtract)
            V.tensor_tensor(d23[:, :], d23[:, :], rden[:, :], Alu.mult)
            V.tensor_tensor(tp1[:, :], tau1[:, :], tau3[:, :], Alu.subtract)
            V.reciprocal(rden[:, :], tp1[:, :])
            V.tensor_tensor(aq[:, :], d12[:, :], d23[:, :], Alu.subtract)
            V.tensor_tensor(aq[:, :], aq[:, :], rden[:, :], Alu.mult)  # f''/2
            V.tensor_tensor(tp1[:, :], tau3[:, :], tau2[:, :], Alu.subtract)
            V.tensor_tensor(tp1[:, :], tp1[:, :], aq[:, :], Alu.mult)
            V.tensor_tensor(bq_c[:, :], d23[:, :], tp1[:, :], Alu.add)  # f'(tau3)
            V.tensor_scalar(cq[:, :], f3[:, :], -1.0, None, Alu.add)  # f3 - 1
            # disc = b^2 - 4ac ; sq = sqrt(max(disc,0))
            V.tensor_tensor(tp1[:, :], bq_c[:, :], bq_c[:, :], Alu.mult)
            V.tensor_tensor(tp2[:, :], aq[:, :], cq[:, :], Alu.mult)
            V.scalar_tensor_tensor(
                tp1[:, :], tp2[:, :], -4.0, tp1[:, :], Alu.mult, Alu.add
            )
            V.tensor_scalar(tp1[:, :], tp1[:, :], 0.0, None, Alu.max)
            S.activation(sq_[:, :], tp1[:, :], Act.Sqrt)
            # den = bq<0 ? bq - sq : bq + sq   == bq + sq - 2*mask*sq
            V.tensor_scalar(tp1[:, :], bq_c[:, :], 0.0, None, Alu.is_lt)
            V.tensor_tensor(tp1[:, :], tp1[:, :], sq_[:, :], Alu.mult)
            V.tensor_tensor(tp2[:, :], bq_c[:, :], sq_[:, :], Alu.add)
            V.scalar_tensor_tensor(
                tp2[:, :], tp1[:, :], -2.0, tp2[:, :], Alu.mult, Alu.add
            )
            # keep |den| away from 0 (sign-preserving):
            V.tensor_scalar(tp1[:, :], tp2[:, :], 0.0, None, Alu.is_ge)
            V.tensor_scalar(tp1[:, :], tp1[:, :], 2.0, -1.0, Alu.mult, Alu.add)  # sign
            V.tensor_scalar(tp1[:, :], tp1[:, :], 1e-9, None, Alu.mult)
            V.tensor_tensor(tp2[:, :], tp2[:, :], tp1[:, :], Alu.add)
            V.reciprocal(rden[:, :], tp2[:, :])
            # tau4 = tau3 - 2*c/den, clamped to [LO, HI]
            V.tensor_tensor(tp1[:, :], cq[:, :], rden[:, :], Alu.mult)
            V.scalar_tensor_tensor(
                tau4[:, :], tp1[:, :], -2.0, tau3[:, :], Alu.mult, Alu.add
            )
            V.tensor_tensor(tau4[:, :], tau4[:, :], lo_[:, :], Alu.max)
            V.tensor_tensor(tau4[:, :], tau4[:, :], hi_[:, :], Alu.min)

            # ---- output pass from fp16 z: out = relu(z - tau)^2 (fp32) ----
            for j in range(NT):
                t16 = work16.tile([P, N], F16, tag="T16", name="t16o")
                V.tensor_scalar(
                    t16[:, :], z16[:, j, :], tau4[:, j : j + 1], 0.0,
                    Alu.subtract, Alu.max,
                )
                ou = outp.tile([P, N], F32, tag="ou", name="ou")
                if j % 3 == 2:
                    V.scalar_tensor_tensor(
                        ou[:, :], t16[:, :], 0.0, t16[:, :], Alu.add, Alu.mult
                    )
                else:
                    S.activation(ou[:, :], t16[:, :], Act.Square)
                SY.dma_start(out=out_d[j * P : (j + 1) * P, :], in_=ou[:, :])

    nc.compile()
    return nc


def _get_nc() -> bass.Bass:
    if "nc" not in _CACHE:
        _CACHE["nc"] = _build_nc()
    return _CACHE["nc"]


def _run(in_maps, trace=False, **kw):
    nc = _get_nc()
    return run_bass_kernel_spmd(
        nc, in_maps, core_ids=list(range(B)), trace=trace, **kw
    )


def _make_in_maps(x_c, x_n, Wq, bq, Wk, bk):
    x_c = np.ascontiguousarray(np.asarray(x_c, dtype=np.float32))
    x_n = np.ascontiguousarray(np.asarray(x_n, dtype=np.float32))
    Wq = np.ascontiguousarray(np.asarray(Wq, dtype=np.float32))
    Wk = np.ascontiguousarray(np.asarray(Wk, dtype=np.float32))
    bq = np.ascontiguousarray(np.asarray(bq, dtype=np.float32).reshape(D, 1))
    bk = np.ascontiguousarray(np.asarray(bk, dtype=np.float32).reshape(D, 1))
    return [
        {
            "x_c": x_c[i],
            "x_n": x_n[i],
            "Wq": Wq,
            "bq": bq,
            "Wk": Wk,
            "bk": bk,
        }
        for i in range(B)
    ]


def kernel(x_c, x_n, Wq, bq, Wk, bk):
    res = _run(_make_in_maps(x_c, x_n, Wq, bq, Wk, bk))
    out = np.stack([res.results[i]["out"] for i in range(B)], axis=0)
    return out.astype(np.float32)


if __name__ == "__main__":
    rng = np.random.default_rng(0)
    s = float(1.0 / np.sqrt(D))
    inputs = {
        "x_c": rng.standard_normal((B, N, D)).astype(np.float32),
        "x_n": rng.standard_normal((B, N, D)).astype(np.float32),
        "Wq": rng.uniform(-s, s, (D, D)).astype(np.float32),
        "bq": rng.uniform(-s, s, (D,)).astype(np.float32),
        "Wk": rng.uniform(-s, s, (D, D)).astype(np.float32),
        "bk": rng.uniform(-s, s, (D,)).astype(np.float32),
    }
    out = kernel(**inputs)
    print("out", out.shape, out.dtype, float(out.max()))


# revision 12
# speedup vs baseline: 1.2662x; 1.0653x over previous
"""Trainium2 Bass kernel for cross-attention + entmax15 (sparse attention scores).

Computes, per batch b:
    Q = x_c[b] @ Wq.T + bq ; K = x_n[b] @ Wk.T + bk
    A = Q @ K.T / sqrt(128) ; out[b] = entmax15(A)   (exact 1.5-entmax, row-wise)

Strategy: data-parallel over batch across 8 NeuronCores (B == 8 -> one batch
per core). entmax15 is computed without sorting: the threshold tau* per row
solves f(tau) = sum_i relu(z_i - tau)^2 = 1 (z = A/2).  f is convex,
decreasing, piecewise quadratic in tau.  We initialize tau analytically from
per-row Gaussian moment estimates, refine with three f-only evaluations
(bracketed secant steps), then solve the local quadratic through the last
three (tau, f) points (Muller step — exact on a fixed support piece).
Finally out = relu(z - tau)^2.
"""

import sys

sys.path.insert(0, "/opt/trn_rl_repo")

import numpy as np

import concourse.bass as bass
import concourse.mybir as mybir
from concourse import bacc
from concourse.bass_utils import run_bass_kernel_spmd
from concourse.masks import make_identity
from concourse.tile import TileContext

B, N, D = 8, 2048, 128
P = 128
NT = N // P  # 16 row-tiles of 128 rows per core
SC = float(1.0 / (2.0 * np.sqrt(np.float64(D))))  # folds /sqrt(D) and /2 into Q
BLK = 256  # columns used for cheap per-row variance / max estimates
MARGIN = 0.02
DOWNCLIP = 0.08
GINIT_STEPS = 3
C1 = float(1.0 / np.sqrt(2.0 * np.pi))
# Zelen & Severo (A&S 26.2.16) rational approx of the normal tail:
# Phic(t) ~= phi(t) * (ZB1*k + ZB2*k^2 + ZB3*k^3), k = 1/(1+ZP*t)
ZB1, ZB2, ZB3, ZP = 0.4361836, -0.1201676, 0.9372980, 0.33267

F32 = mybir.dt.float32
F16 = mybir.dt.float16
BF16 = mybir.dt.bfloat16
Alu = mybir.AluOpType
Act = mybir.ActivationFunctionType

_CACHE = {}


def _build_nc() -> bass.Bass:
    nc = bacc.Bacc(None, target_bir_lowering=False)
    xc_d = nc.dram_tensor("x_c", [N, D], F32, kind="ExternalInput")
    xn_d = nc.dram_tensor("x_n", [N, D], F32, kind="ExternalInput")
    wq_d = nc.dram_tensor("Wq", [D, D], F32, kind="ExternalInput")
    bq_d = nc.dram_tensor("bq", [D, 1], F32, kind="ExternalInput")
    wk_d = nc.dram_tensor("Wk", [D, D], F32, kind="ExternalInput")
    bk_d = nc.dram_tensor("bk", [D, 1], F32, kind="ExternalInput")
    out_d = nc.dram_tensor("out", [N, N], F32, kind="ExternalOutput")

    V = nc.vector
    S = nc.scalar
    TE = nc.tensor
    SY = nc.sync

    with TileContext(nc) as tc:
        with (
            tc.tile_pool(name="consts", bufs=1) as consts,
            tc.tile_pool(name="persist", bufs=1) as persist,
            tc.tile_pool(name="stats", bufs=1) as stats,
            tc.tile_pool(name="work16", bufs=3) as work16,
            tc.tile_pool(name="junk16", bufs=3) as junk16,
            tc.tile_pool(name="outp", bufs=3) as outp,
            tc.tile_pool(name="ps", bufs=2, space="PSUM") as ps,
        ):
            ident = consts.tile([P, P], F32, tag="ident")
            make_identity(nc, ident)

            # ---- biases ----
            bq_sb = consts.tile([P, 1], F32, tag="bq")
            bk_sb = consts.tile([P, 1], F32, tag="bk")
            SY.dma_start(out=bq_sb[:, :], in_=bq_d[:, :])
            SY.dma_start(out=bk_sb[:, :], in_=bk_d[:, :])
            bqs = consts.tile([P, 1], F32, tag="bqs")
            V.tensor_scalar(bqs[:, :], bq_sb[:, :], SC, None, Alu.mult)

            # ---- weights + their transposes (lhsT = W^T) ----
            wq_sb = consts.tile([P, P], F32, tag="wq")
            wk_sb = consts.tile([P, P], F32, tag="wk")
            SY.dma_start(out=wq_sb[:, :], in_=wq_d[:, :])
            SY.dma_start(out=wk_sb[:, :], in_=wk_d[:, :])
            wqT = consts.tile([P, P], F32, tag="wqT")
            wkT = consts.tile([P, P], F32, tag="wkT")
            wt_ps = ps.tile([P, 2, P], F32, tag="ps")
            TE.transpose(wt_ps[:, 0, :], wq_sb[:, :], ident[:, :])
            TE.transpose(wt_ps[:, 1, :], wk_sb[:, :], ident[:, :])
            V.tensor_copy(wqT[:, :], wt_ps[:, 0, :])
            V.tensor_copy(wkT[:, :], wt_ps[:, 1, :])

            # ---- load x_c, x_n (4 chunked DMAs each) and transpose to [e, n]
            xc_sb = persist.tile([P, NT, P], F32, tag="xc_sb")
            xn_sb = persist.tile([P, NT, P], F32, tag="xn_sb")
            xcT = persist.tile([P, NT, P], F32, tag="xcT")  # x_c^T  [e, n]
            xnT = persist.tile([P, NT, P], F32, tag="xnT")  # x_n^T  [e, n]
            for src_d, stage, dstT in ((xn_d, xn_sb, xnT), (xc_d, xc_sb, xcT)):
                src_r = src_d.rearrange("(t p) e -> p t e", p=P)
                for c in range(4):
                    SY.dma_start(
                        out=stage[:, 4 * c : 4 * c + 4, :],
                        in_=src_r[:, 4 * c : 4 * c + 4, :],
                    )
                x_ps = ps.tile([P, NT, P], F32, tag="ps")
                for j in range(NT):
                    TE.transpose(x_ps[:, j, :], stage[:, j, :], ident[:, :])
                S.activation(dstT[:, :, :], x_ps[:, :, :], Act.Copy)

            # ---- projections: Q'^T = (Wq x_c^T + bq) * SC, K'^T = Wk x_n^T + bk
            QTb = persist.tile([P, N], BF16, tag="QTb")
            KTb = persist.tile([P, N], BF16, tag="KTb")
            for (wT, xT, dst, bias_ap, scale) in (
                (wkT, xnT, KTb, bk_sb, 1.0),
                (wqT, xcT, QTb, bqs, SC),
            ):
                pr_ps = ps.tile([P, N], F32, tag="ps")
                for mb in range(4):
                    TE.matmul(
                        pr_ps[:, mb * 512 : (mb + 1) * 512],
                        lhsT=wT[:, :],
                        rhs=xT[:, 4 * mb : 4 * mb + 4, :],
                        start=True,
                        stop=True,
                    )
                    # chunked copy so downstream matmuls can start per-block
                    S.activation(
                        dst[:, mb * 512 : (mb + 1) * 512],
                        pr_ps[:, mb * 512 : (mb + 1) * 512],
                        Act.Identity, bias=bias_ap[:, :], scale=scale,
                    )

            # ---- per-row stat tiles [P, NT] fp32 ----
            def st(tag):
                return stats.tile([P, NT], F32, tag=tag, name=tag)

            mu_a, s2_a, msub = st("mu"), st("s2"), st("msub")
            var, sig, t_ = st("var"), st("sig"), st("t")
            tsq, e_, r_, phic = st("tsq"), st("e"), st("r"), st("phic")
            w_, f_f, tp1, tp2 = st("w"), st("ff"), st("tp1"), st("tp2")
            ns2, rden, gmod = st("ns2"), st("rden"), st("gmod")
            lo_, hi_ = st("lo"), st("hi")
            ab = st("ab")
            tau1, tau2, tau3, tau4 = st("tau1"), st("tau2"), st("tau3"), st("tau4")
            f1, f2, f3 = st("f1"), st("f2"), st("f3")
            d12, d23, aq, bq_c, cq = st("d12"), st("d23"), st("aq"), st("bqc"), st("cq")
            mid, sq_ = st("mid"), st("sq")

            # ---- z pass: z = Q'^T.T @ K'^T ; keep fp16 copy + row stats ----
            z16 = persist.tile([P, NT, N], F16, tag="z16")
            for j in range(NT):
                z_ps = ps.tile([P, N], F32, tag="ps")
                for mb in range(4):
                    TE.matmul(
                        z_ps[:, mb * 512 : (mb + 1) * 512],
                        lhsT=QTb[:, j * P : (j + 1) * P],
                        rhs=KTb[:, mb * 512 : (mb + 1) * 512],
                        start=True,
                        stop=True,
                    )
                if j % 3 == 2:
                    # DVE copy+cast with mean accumulate (op1 = reduce op)
                    V.tensor_scalar(
                        z16[:, j, :], z_ps[:, :], 0.0, None, Alu.add, Alu.add,
                        accum_out=mu_a[:, j : j + 1],
                    )
                else:
                    S.activation(
                        z16[:, j, :], z_ps[:, :], Act.Identity,
                        accum_out=mu_a[:, j : j + 1],
                    )
                # block stats on the fp16 copy
                V.tensor_reduce(
                    msub[:, j : j + 1], z16[:, j, 0:BLK], mybir.AxisListType.X, Alu.max
                )
                jk = junk16.tile([P, BLK], F16, tag="jkb")
                V.scalar_tensor_tensor(
                    jk[:, :], z16[:, j, 0:BLK], 0.0, z16[:, j, 0:BLK],
                    Alu.add, Alu.mult,
                    accum_out=s2_a[:, j : j + 1],
                )

            # ---- Gaussian-moment analytic init for tau ----
            V.tensor_scalar(mu_a[:, :], mu_a[:, :], 1.0 / N, None, Alu.mult)
            V.tensor_scalar(s2_a[:, :], s2_a[:, :], 1.0 / BLK, None, Alu.mult)
            V.tensor_tensor(tp1[:, :], mu_a[:, :], mu_a[:, :], Alu.mult)
            V.tensor_tensor(var[:, :], s2_a[:, :], tp1[:, :], Alu.subtract)
            V.tensor_scalar(var[:, :], var[:, :], 1e-12, None, Alu.max)
            S.activation(sig[:, :], var[:, :], Act.Sqrt)
            # ns2 = N * var * C1 (C1 folded so phi == exp(-t^2/2) below)
            V.tensor_scalar(ns2[:, :], var[:, :], float(N) * C1, None, Alu.mult)
            # Solve N*sig^2*F(t) = 1,  F(t) = (1+t^2)*Phic(t) - t*phi(t), by
            # Newton in t, with Phic via the Zelen-Severo rational approx.
            V.memset(t_[:, :], 2.0)
            for gi in range(GINIT_STEPS + 1):
                V.tensor_tensor(tsq[:, :], t_[:, :], t_[:, :], Alu.mult)
                S.activation(e_[:, :], tsq[:, :], Act.Exp, scale=-0.5)  # phi/C1
                V.tensor_scalar(tp1[:, :], t_[:, :], ZP, 1.0, Alu.mult, Alu.add)
                V.reciprocal(r_[:, :], tp1[:, :])  # k = 1/(1+ZP*t)
                V.tensor_scalar(phic[:, :], r_[:, :], ZB3, ZB2, Alu.mult, Alu.add)
                V.tensor_tensor(phic[:, :], phic[:, :], r_[:, :], Alu.mult)
                V.tensor_scalar(phic[:, :], phic[:, :], ZB1, None, Alu.add)
                V.tensor_tensor(phic[:, :], phic[:, :], r_[:, :], Alu.mult)  # Rk
                # dF/(2 C1) = (t*Rk - 1) * e
                V.tensor_tensor(tp2[:, :], t_[:, :], phic[:, :], Alu.mult)
                V.tensor_scalar(tp2[:, :], tp2[:, :], -1.0, None, Alu.add)
                V.tensor_tensor(tp2[:, :], tp2[:, :], e_[:, :], Alu.mult)
                if gi == GINIT_STEPS:
                    # analytic slope model: g(tau) = -N*sig*C1*(t*Rk-1)*e
                    V.tensor_tensor(gmod[:, :], tp2[:, :], sig[:, :], Alu.mult)
                    V.tensor_scalar(
                        gmod[:, :], gmod[:, :], -float(N) * C1, None, Alu.mult
                    )
                    V.tensor_scalar(gmod[:, :], gmod[:, :], 1e-6, None, Alu.max)
                    break
                V.tensor_scalar(w_[:, :], tsq[:, :], 1.0, None, Alu.add)  # 1+t^2
                V.tensor_tensor(f_f[:, :], w_[:, :], phic[:, :], Alu.mult)
                V.tensor_tensor(f_f[:, :], f_f[:, :], t_[:, :], Alu.subtract)
                V.tensor_tensor(f_f[:, :], f_f[:, :], e_[:, :], Alu.mult)  # F/C1
                # num = ns2*F - 1 ; den = ns2*(dF/2) ; t -= num/(2*den)
                V.tensor_tensor(tp1[:, :], f_f[:, :], ns2[:, :], Alu.mult)
                V.tensor_scalar(tp1[:, :], tp1[:, :], -1.0, None, Alu.add)
                V.tensor_tensor(tp2[:, :], tp2[:, :], ns2[:, :], Alu.mult)
                V.reciprocal(rden[:, :], tp2[:, :])
                V.tensor_tensor(tp1[:, :], tp1[:, :], rden[:, :], Alu.mult)
                V.scalar_tensor_tensor(
                    t_[:, :], tp1[:, :], -0.5, t_[:, :], Alu.mult, Alu.add
                )
                V.tensor_scalar(t_[:, :], t_[:, :], 0.0, 6.0, Alu.max, Alu.min)
            # tau1 = max(mu + sig*t - MARGIN, msub - 1)
            V.tensor_tensor(tp1[:, :], sig[:, :], t_[:, :], Alu.mult)
            V.tensor_tensor(tau1[:, :], mu_a[:, :], tp1[:, :], Alu.add)
            V.tensor_scalar(tau1[:, :], tau1[:, :], -MARGIN, None, Alu.add)
            V.tensor_scalar(lo_[:, :], msub[:, :], -1.0, None, Alu.add)
            V.tensor_scalar(hi_[:, :], msub[:, :], 1.0, None, Alu.add)
            V.tensor_tensor(tau1[:, :], tau1[:, :], lo_[:, :], Alu.max)

            # ---- f-only evaluation: f(tau) = sum relu(z16 - tau)^2 ----
            def eval_f(tau_ap, f_ap):
                for j in range(NT):
                    t16 = work16.tile([P, N], F16, tag="T16", name="t16")
                    V.tensor_scalar(
                        t16[:, :], z16[:, j, :], tau_ap[:, j : j + 1], 0.0,
                        Alu.subtract, Alu.max,
                    )
                    jq = junk16.tile([P, N], F16, tag="jq", name="jq")
                    if j % 3 == 2:
                        V.scalar_tensor_tensor(
                            jq[:, :], t16[:, :], 0.0, t16[:, :], Alu.add, Alu.mult,
                            accum_out=f_ap[:, j : j + 1],
                        )
                    else:
                        S.activation(
                            jq[:, :], t16[:, :], Act.Square,
                            accum_out=f_ap[:, j : j + 1],
                        )

            def update_brackets(tau_ap, f_ap):
                # ab = f >= 1 ; LO = max(LO, tau - 4*(1-ab)) ; HI = min(HI, tau + 4*ab)
                V.tensor_scalar(ab[:, :], f_ap[:, :], 1.0, None, Alu.is_ge)
                V.tensor_scalar(tp1[:, :], ab[:, :], -1.0, None, Alu.add)
                V.scalar_tensor_tensor(
                    tp1[:, :], tp1[:, :], 4.0, tau_ap[:, :], Alu.mult, Alu.add
                )
                V.tensor_tensor(lo_[:, :], lo_[:, :], tp1[:, :], Alu.max)
                V.scalar_tensor_tensor(
                    tp2[:, :], ab[:, :], 4.0, tau_ap[:, :], Alu.mult, Alu.add
                )
                V.tensor_tensor(hi_[:, :], hi_[:, :], tp2[:, :], Alu.min)

            def guarded_step(tau_ap, f_ap, slope_ap, out_ap):
                # tn = tau + max((f-1)/slope, -DOWNCLIP)
                # out = ab ? min(tn, HI) : max(tn, mid)
                V.reciprocal(rden[:, :], slope_ap[:, :])
                V.tensor_scalar(tp2[:, :], f_ap[:, :], -1.0, None, Alu.add)
                V.tensor_tensor(tp2[:, :], tp2[:, :], rden[:, :], Alu.mult)
                V.tensor_scalar(tp2[:, :], tp2[:, :], -DOWNCLIP, None, Alu.max)
                V.tensor_tensor(tp2[:, :], tau_ap[:, :], tp2[:, :], Alu.add)
                V.tensor_tensor(mid[:, :], lo_[:, :], hi_[:, :], Alu.add)
                V.tensor_scalar(mid[:, :], mid[:, :], 0.5, None, Alu.mult)
                V.tensor_tensor(tp1[:, :], tp2[:, :], hi_[:, :], Alu.min)  # above
                V.tensor_tensor(tp2[:, :], tp2[:, :], mid[:, :], Alu.max)  # below
                V.tensor_tensor(tp1[:, :], tp1[:, :], tp2[:, :], Alu.subtract)
                V.tensor_tensor(tp1[:, :], tp1[:, :], ab[:, :], Alu.mult)
                V.tensor_tensor(out_ap[:, :], tp2[:, :], tp1[:, :], Alu.add)

            def force_distinct(tau_ap, other_ap, off=-1e-4):
                # if |tau - other| < 1e-5: tau = other + off
                V.tensor_tensor(tp1[:, :], tau_ap[:, :], other_ap[:, :], Alu.subtract)
                S.activation(tp1[:, :], tp1[:, :], Act.Abs)
                V.tensor_scalar(tp1[:, :], tp1[:, :], 1e-5, None, Alu.is_lt)
                # tau = tau*(1-m) + (other+off)*m  ==  tau + m*(other+off-tau)
                V.tensor_scalar(tp2[:, :], other_ap[:, :], off, None, Alu.add)
                V.tensor_tensor(tp2[:, :], tp2[:, :], tau_ap[:, :], Alu.subtract)
                V.tensor_tensor(tp2[:, :], tp2[:, :], tp1[:, :], Alu.mult)
                V.tensor_tensor(tau_ap[:, :], tau_ap[:, :], tp2[:, :], Alu.add)

            # eval 1: Newton step with analytic slope 2*g_model
            eval_f(tau1, f1)
            update_brackets(tau1, f1)
            V.tensor_scalar(tp1[:, :], gmod[:, :], 2.0, None, Alu.mult)
            V.tensor_copy(sq_[:, :], tp1[:, :])  # slope buffer
            guarded_step(tau1, f1, sq_, tau2)
            force_distinct(tau2, tau1)

            # eval 2: secant slope from (tau1,f1)
            eval_f(tau2, f2)
            update_brackets(tau2, f2)
            # d12 = (f1-f2)/(tau1-tau2)  (negative); slope = max(-d12, eps->gmod)
            V.tensor_tensor(tp1[:, :], tau1[:, :], tau2[:, :], Alu.subtract)
            V.reciprocal(rden[:, :], tp1[:, :])
            V.tensor_tensor(d12[:, :], f1[:, :], f2[:, :], Alu.subtract)
            V.tensor_tensor(d12[:, :], d12[:, :], rden[:, :], Alu.mult)
            V.tensor_scalar(sq_[:, :], d12[:, :], -1.0, None, Alu.mult)
            V.tensor_scalar(sq_[:, :], sq_[:, :], 1e-6, None, Alu.max)
            guarded_step(tau2, f2, sq_, tau3)
            force_distinct(tau3, tau2, off=-1.3e-4)
            force_distinct(tau3, tau1, off=-2.1e-4)

            # eval 3 + Muller quadratic solve through the three points
            eval_f(tau3, f3)
            update_brackets(tau3, f3)
            V.tensor_tensor(tp1[:, :], tau2[:, :], tau3[:, :], Alu.subtract)
            V.reciprocal(rden[:, :], tp1[:, :])
            V.tensor_tensor(d23[:, :], f2[:, :], f3[:, :], Alu.subtract)
            V.tensor_tensor(d23[:, :], d23[:, :], rden[:, :], Alu.mult)
            V.tensor_tensor(tp1[:, :], tau1[:, :], tau3[:, :], Alu.subtract)
            V.reciprocal(rden[:, :], tp1[:, :])
            V.tensor_tensor(aq[:, :], d12[:, :], d23[:, :], Alu.subtract)
            V.tensor_tensor(aq[:, :], aq[:, :], rden[:, :], Alu.mult)  # f''/2
            V.tensor_tensor(tp1[:, :], tau3[:, :], tau2[:, :], Alu.subtract)
            V.tensor_tensor(tp1[:, :], tp1[:, :], aq[:, :], Alu.mult)
            V.tensor_tensor(bq_c[:, :], d23[:, :], tp1[:, :], Alu.add)  # f'(tau3)
            V.tensor_scalar(cq[:, :], f3[:, :], -1.0, None, Alu.add)  # f3 - 1
            # disc = b^2 - 4ac ; sq = sqrt(max(disc,0))
            V.tensor_tensor(tp1[:, :], bq_c[:, :], bq_c[:, :], Alu.mult)
            V.tensor_tensor(tp2[:, :], aq[:, :], cq[:, :], Alu.mult)
            V.scalar_tensor_tensor(
                tp1[:, :], tp2[:, :], -4.0, tp1[:, :], Alu.mult, Alu.add
            )
            V.tensor_scalar(tp1[:, :], tp1[:, :], 0.0, None, Alu.max)
            S.activation(sq_[:, :], tp1[:, :], Act.Sqrt)
            # den = bq<0 ? bq - sq : bq + sq   == bq + sq - 2*mask*sq
            V.tensor_scalar(tp1[:, :], bq_c[:, :], 0.0, None, Alu.is_lt)
            V.tensor_tensor(tp1[:, :], tp1[:, :], sq_[:, :], Alu.mult)
            V.tensor_tensor(tp2[:, :], bq_c[:, :], sq_[:, :], Alu.add)
            V.scalar_tensor_tensor(
                tp2[:, :], tp1[:, :], -2.0, tp2[:, :], Alu.mult, Alu.add
            )
            # keep |den| away from 0 (sign-preserving):
            V.tensor_scalar(tp1[:, :], tp2[:, :], 0.0, None, Alu.is_ge)
            V.tensor_scalar(tp1[:, :], tp1[:, :], 2.0, -1.0, Alu.mult, Alu.add)  # sign
            V.tensor_scalar(tp1[:, :], tp1[:, :], 1e-9, None, Alu.mult)
            V.tensor_tensor(tp2[:, :], tp2[:, :], tp1[:, :], Alu.add)
            V.reciprocal(rden[:, :], tp2[:, :])
            # tau4 = tau3 - 2*c/den, clamped to [LO, HI]
            V.tensor_tensor(tp1[:, :], cq[:, :], rden[:, :], Alu.mult)
            V.scalar_tensor_tensor(
                tau4[:, :], tp1[:, :], -2.0, tau3[:, :], Alu.mult, Alu.add
            )
            V.tensor_tensor(tau4[:, :], tau4[:, :], lo_[:, :], Alu.max)
            V.tensor_tensor(tau4[:, :], tau4[:, :], hi_[:, :], Alu.min)

            # ---- output pass from fp16 z: out = relu(z - tau)^2 (fp32) ----
            for j in range(NT):
                t16 = work16.tile([P, N], F16, tag="T16", name="t16o")
                V.tensor_scalar(
                    t16[:, :], z16[:, j, :], tau4[:, j : j + 1], 0.0,
                    Alu.subtract, Alu.max,
                )
                ou = outp.tile([P, N], F32, tag="ou", name="ou")
                if j % 3 == 2:
                    V.scalar_tensor_tensor(
                        ou[:, :], t16[:, :], 0.0, t16[:, :], Alu.add, Alu.mult
                    )
                else:
                    S.activation(ou[:, :], t16[:, :], Act.Square)
                SY.dma_start(out=out_d[j * P : (j + 1) * P, :], in_=ou[:, :])

    nc.compile()
    return nc


def _get_nc() -> bass.Bass:
    if "nc" not in _CACHE:
        _CACHE["nc"] = _build_nc()
    return _CACHE["nc"]


def _run(in_maps, trace=False, **kw):
    nc = _get_nc()
    return run_bass_kernel_spmd(
        nc, in_maps, core_ids=list(range(B)), trace=trace, **kw
    )


def _make_in_maps(x_c, x_n, Wq, bq, Wk, bk):
    x_c = np.ascontiguousarray(np.asarray(x_c, dtype=np.float32))
    x_n = np.ascontiguousarray(np.asarray(x_n, dtype=np.float32))
    Wq = np.ascontiguousarray(np.asarray(Wq, dtype=np.float32))
    Wk = np.ascontiguousarray(np.asarray(Wk, dtype=np.float32))
    bq = np.ascontiguousarray(np.asarray(bq, dtype=np.float32).reshape(D, 1))
    bk = np.ascontiguousarray(np.asarray(bk, dtype=np.float32).reshape(D, 1))
    return [
        {
            "x_c": x_c[i],
            "x_n": x_n[i],
            "Wq": Wq,
            "bq": bq,
            "Wk": Wk,
            "bk": bk,
        }
        for i in range(B)
    ]


def kernel(x_c, x_n, Wq, bq, Wk, bk):
    res = _run(_make_in_maps(x_c, x_n, Wq, bq, Wk, bk))
    out = np.stack([res.results[i]["out"] for i in range(B)], axis=0)
    return out.astype(np.float32)


if __name__ == "__main__":
    rng = np.random.default_rng(0)
    s = float(1.0 / np.sqrt(D))
    inputs = {
        "x_c": rng.standard_normal((B, N, D)).astype(np.float32),
        "x_n": rng.standard_normal((B, N, D)).astype(np.float32),
        "Wq": rng.uniform(-s, s, (D, D)).astype(np.float32),
        "bq": rng.uniform(-s, s, (D,)).astype(np.float32),
        "Wk": rng.uniform(-s, s, (D, D)).astype(np.float32),
        "bk": rng.uniform(-s, s, (D,)).astype(np.float32),
    }
    out = kernel(**inputs)
    print("out", out.shape, out.dtype, float(out.max()))
